# revision 17
# baseline (speedup 1.0000x reference)
"""Trainium2 Bass kernel for nn_Enhancer_63350767616202.

Data-parallel over batch (8 samples -> 8 cores). Per core, everything stays in
channel-major [C, T] layout (matches the input's [C, H, W] memory layout), so
the two big linear layers, the partial 3x3 conv and the depthwise 3x3 conv all
run on the tensor engine with zero transposes:

  phase 1 (streamed over 12 row-blocks of 8 rows):
    LN stats via ones-matmul reductions (float32r, M=128 -> broadcast for free)
    y = (x - mu) * rsqrt(var+eps)        [ln gamma/beta folded into weights]
    pconv   : 9 accumulating matmuls over a zero-padded fp8 plane
    linear1 : bf16 matmuls, Gelu eviction on ACT (h1 -> padded fp8 windows)
    dwconv  : fp8 DoubleRow diagonal matmuls (2 taps per matmul)
    linear2 : bf16 matmuls over gelu(dwconv)*h2, evicted to a DRAM scratch
    channel sums of x and mlp accumulated on the fly (accum_out)
  phase 2: SplitAttn tail on [256]-vectors (fc1 -> LN -> relu -> fc2 -> sigmoid)
  phase 3: out = (x + mlp) * a   (restreamed)
"""

import os
import sys

for _p in ("/opt/trn_rl_repo", "/root/.axon_site/_ro/trn_rl_repo"):
    if os.path.isdir(_p) and _p not in sys.path:
        sys.path.append(_p)

import numpy as np
import ml_dtypes

import concourse.bass as bass
import concourse.mybir as mybir
import concourse.tile as tile
from concourse import bacc
from concourse.tile import TileContext

F32 = mybir.dt.float32
F32R = mybir.dt.float32r
BF16 = mybir.dt.bfloat16
FP8 = mybir.dt.float8e4
AF = mybir.ActivationFunctionType
OP = mybir.AluOpType

NPBF16 = ml_dtypes.bfloat16
NPFP8 = ml_dtypes.float8_e4m3

C = 256
H, W = 96, 192
T = H * W
HID = 512
F1 = 1024
DC = 64          # partial conv channels
LN_EPS = 1e-5

RB = 8           # rows per block
TB = RB * W      # tokens per block (1536)
NB = H // RB     # 12 blocks
QL = 512         # linear-chunk tokens
NQL = TB // QL   # 3
QC = 384         # conv-chunk tokens (2 rows)
NQC = TB // QC   # 4
WP = W + 2       # padded width 194
ZR = H + 2       # padded rows for pconv plane
WINR = RB + 3    # padded rows per window (8 + halo x2 + junk row)

PSCALE = 64.0    # fp8 weight scaling for pconv
DSCALE = 64.0    # fp8 weight scaling for dwconv

# dwconv DoubleRow tap pairs: ((dy0,dx0),(dy1,dx1)|None). The pair delta is
# always +WP (one padded row down): the device crashes on negative or
# overlapping pair strides. Singles carry a zero-weight second tap that reads
# the zeroed junk row below the window.
DW_PAIRS = [
    (((-1, -1), (0, -1)), WP),
    (((-1, 0), (0, 0)), WP),
    (((-1, 1), (0, 1)), WP),
    (((1, -1), None), WP),
    (((1, 0), None), WP),
    (((1, 1), None), WP),
]

N_CORES = 8


def _ap(base, offset_delta, ap_dims):
    """Build a raw AP on base's tensor with extra offset and explicit dims."""
    return bass.AP(tensor=base.tensor, offset=base.offset + offset_delta,
                   ap=ap_dims)


def build_bass():
    nc = bacc.Bacc("TRN2", target_bir_lowering=False, debug=False,
                   num_devices=N_CORES)

    # ---- per-core I/O ----
    x_d = nc.dram_tensor("x", [C, H, W], F32, kind="ExternalInput")
    w1_d = nc.dram_tensor("w1t", [C, F1], BF16, kind="ExternalInput")
    b1_d = nc.dram_tensor("b1", [F1, 1], F32, kind="ExternalInput")
    w2_d = nc.dram_tensor("w2t", [HID, C], BF16, kind="ExternalInput")
    b2_d = nc.dram_tensor("b2", [C, 1], F32, kind="ExternalInput")
    pw_d = nc.dram_tensor("pw", [9, DC, DC], FP8, kind="ExternalInput")
    pc_d = nc.dram_tensor("pconst", [DC, 1], F32, kind="ExternalInput")
    dw_d = nc.dram_tensor("dwdr", [6, 4, 128, 2 * 128], FP8, kind="ExternalInput")
    db_d = nc.dram_tensor("dwb", [HID, 1], F32, kind="ExternalInput")
    f1_d = nc.dram_tensor("fc1t", [C, C], F32, kind="ExternalInput")
    f2_d = nc.dram_tensor("fc2t", [C, C], F32, kind="ExternalInput")
    bg_d = nc.dram_tensor("bn1g", [1, C], F32, kind="ExternalInput")
    bb_d = nc.dram_tensor("bn1b", [1, C], F32, kind="ExternalInput")
    out_d = nc.dram_tensor("out", [C, H, W], F32, kind="ExternalOutput")

    xf = x_d[:].rearrange("c h w -> c (h w)")
    outf = out_d[:].rearrange("c h w -> c (h w)")

    with TileContext(nc) as tc:
        _build_body(nc, tc, xf, outf, w1_d, b1_d, w2_d, b2_d, pw_d, pc_d,
                    dw_d, db_d, f1_d, f2_d, bg_d, bb_d)

    nc.compile()
    return nc


_PERM_POOL = {}


def _tile(tc, shape, dtype, name):
    pool = _PERM_POOL.get(id(tc))
    if pool is None:
        pool = tc.alloc_tile_pool(name="perm", bufs=1)
        _PERM_POOL[id(tc)] = pool
    return pool.tile(shape, dtype, name=name, tag=name)


def _build_body(nc, tc, xf, outf, w1_d, b1_d, w2_d, b2_d, pw_d, pc_d,
                dw_d, db_d, f1_d, f2_d, bg_d, bb_d):
    act, dve, pool_e, te, sdma = nc.scalar, nc.vector, nc.gpsimd, nc.tensor, nc.sync
    S3L = int(os.environ.get("K_S3", "4"))

    # ---------------- persistent tiles ----------------
    w1_sb = [_tile(tc, [128, F1], BF16, name=f"w1_{i}") for i in range(2)]
    w2_sb = [_tile(tc, [128, C], BF16, name=f"w2_{i}") for i in range(4)]
    pw_sb = [_tile(tc, [DC, DC], FP8, name=f"pw_{t}") for t in range(9)]
    dw_sb = [[_tile(tc, [128, 256], FP8, name=f"dw_{p}_{m}") for m in range(4)]
             for p in range(6)]
    b1_sb = [_tile(tc, [128, 1], F32, name=f"b1_{m}") for m in range(8)]
    b2_sb = [_tile(tc, [128, 1], F32, name=f"b2_{m}") for m in range(2)]
    db_sb = [_tile(tc, [128, 1], F32, name=f"db_{m}") for m in range(4)]
    pc_sb = _tile(tc, [DC, 1], F32, name="pc_sb")
    f1_sb = [_tile(tc, [128, C], F32, name=f"f1_{i}") for i in range(2)]
    f2_sb = [_tile(tc, [128, C], F32, name=f"f2_{i}") for i in range(2)]
    bg_sb = _tile(tc, [1, C], F32, name="bg_sb")
    bb_sb = _tile(tc, [1, C], F32, name="bb_sb")
    ones_b = _tile(tc, [128, 128], BF16, name="ones_b")
    eps_sb = _tile(tc, [128, 1], F32, name="eps_sb")
    zpad = _tile(tc, [DC, ZR * WP], FP8, name="zpad")

    dsum = [_tile(tc, [128, NB * NQL], F32, name=f"dsum{i}") for i in range(2)]
    musum = _tile(tc, [128, NB * NQL], F32, name="musum")
    msum = [_tile(tc, [128, NB * NQC], F32, name=f"msum{i}") for i in range(2)]

    for i in range(2):
        sdma.dma_start(w1_sb[i][:], w1_d[i * 128:(i + 1) * 128, :])
        sdma.dma_start(f1_sb[i][:], f1_d[i * 128:(i + 1) * 128, :])
        sdma.dma_start(f2_sb[i][:], f2_d[i * 128:(i + 1) * 128, :])
        sdma.dma_start(b2_sb[i][:], b2_d[i * 128:(i + 1) * 128, :])
    for i in range(4):
        sdma.dma_start(w2_sb[i][:], w2_d[i * 128:(i + 1) * 128, :])
        sdma.dma_start(db_sb[i][:], db_d[i * 128:(i + 1) * 128, :])
    for t in range(9):
        sdma.dma_start(pw_sb[t][:], pw_d[t, :, :])
    for p in range(6):
        for m in range(4):
            sdma.dma_start(dw_sb[p][m][:], dw_d[p, m, :, :])
    for m in range(8):
        sdma.dma_start(b1_sb[m][:], b1_d[m * 128:(m + 1) * 128, :])
    sdma.dma_start(pc_sb[:], pc_d[:, :])
    sdma.dma_start(bg_sb[:], bg_d[:, :])
    sdma.dma_start(bb_sb[:], bb_d[:, :])
    pool_e.memset(ones_b[:], 1.0)
    pool_e.memset(eps_sb[:], LN_EPS)
    pool_e.memset(zpad[:], 0.0)

    zp3 = zpad[:].rearrange("p (r c) -> p r c", c=WP)

    # ---------------- pools ----------------
    import contextlib
    ctx = contextlib.ExitStack()
    xpool = ctx.enter_context(tc.tile_pool(name="xpool", bufs=2))
    spool = ctx.enter_context(tc.tile_pool(name="spool", bufs=2))
    ypool = ctx.enter_context(tc.tile_pool(name="ypool", bufs=2))
    winpool = ctx.enter_context(tc.tile_pool(name="winpool", bufs=2))
    h2pool = ctx.enter_context(tc.tile_pool(name="h2pool", bufs=2))
    gpool = ctx.enter_context(tc.tile_pool(name="gpool", bufs=2))
    mpool = ctx.enter_context(tc.tile_pool(name="mpool", bufs=2))
    opool = ctx.enter_context(tc.tile_pool(name="opool", bufs=2))
    dpool = ctx.enter_context(tc.tile_pool(name="drampool", bufs=1, space="DRAM"))

    pstat = ctx.enter_context(tc.tile_pool(name="pstat", bufs=1, space="PSUM"))
    pl1 = ctx.enter_context(tc.tile_pool(name="pl1", bufs=2, space="PSUM"))
    pz = ctx.enter_context(tc.tile_pool(name="pz", bufs=1, space="PSUM"))
    pdw = ctx.enter_context(tc.tile_pool(name="pdw", bufs=2, space="PSUM"))
    pml = ctx.enter_context(tc.tile_pool(name="pml", bufs=1, space="PSUM"))

    mlp_d = dpool.tile([C, T], BF16, name="mlp_scratch")

    # state carried across pipeline iterations
    zc0_t, yc1_t = {}, {}            # lin1 rhs tiles per block
    win_t = {}                       # win_t[k] = [4 tiles]
    h2_t = {}                        # h2_t[k] = [4 tiles]

    def stage1(b):
        """LN stats + normalized activations for block b."""
        g0 = b * TB
        xb = [xpool.tile([128, TB], F32, tag=f"x{c}", name=f"xb{c}_{b}")
              for c in range(2)]
        for c in range(2):
            sdma.dma_start(xb[c][:], xf[c * 128:(c + 1) * 128, g0:g0 + TB])

        r_blk = spool.tile([128, TB], BF16, tag="r", name=f"r_{b}")
        d_blk = [spool.tile([128, TB], BF16, tag=f"d{c}", name=f"d{c}_{b}")
                 for c in range(2)]
        zc0 = ypool.tile([128, TB], BF16, tag="zc0", name=f"zc0_{b}")
        yc1 = ypool.tile([128, TB], BF16, tag="yc1", name=f"yc1_{b}")
        zc0_t[b], yc1_t[b] = zc0, yc1

        for q in range(NQL):
            s = slice(q * QL, (q + 1) * QL)
            col = b * NQL + q
            # x^2 (ACT), then ones-matmul stat reductions in f32r
            xcv = [spool.tile([128, QL], BF16, tag=f"cv{c}", name=f"cv{c}_{b}{q}")
                   for c in range(2)]
            xsq = [spool.tile([128, QL], BF16, tag=f"sq{c}", name=f"sq{c}_{b}{q}")
                   for c in range(2)]
            for c in range(2):
                pool_e.tensor_copy(xcv[c][:], xb[c][:, s])
                act.square(xsq[c][:], xb[c][:, s])
            pmu = pstat.tile([128, QL], F32, tag="pmu", name=f"pmu_{b}{q}")
            psq = pstat.tile([128, QL], F32, tag="psq", name=f"psq_{b}{q}")
            for c in range(2):
                te.matmul(pmu[:], ones_b[:], xcv[c][:],
                          start=(c == 0), stop=(c == 1))
                te.matmul(psq[:], ones_b[:], xsq[c][:],
                          start=(c == 0), stop=(c == 1))
            mean = spool.tile([128, QL], BF16, tag="mean", name=f"mean_{b}{q}")
            dve.tensor_scalar(mean[:], pmu[:], 1.0 / C, 0.0, OP.mult,
                              OP.add, accum_out=musum[:, col:col + 1])
            msq = spool.tile([128, QL], BF16, tag="msq", name=f"msq_{b}{q}")
            dve.tensor_mul(msq[:], mean[:], mean[:])
            varv = spool.tile([128, QL], F32, tag="var", name=f"var_{b}{q}")
            dve.scalar_tensor_tensor(varv[:], psq[:], 1.0 / C, msq[:],
                                     OP.mult, OP.subtract)
            lnv = spool.tile([128, QL], F32, tag="lnv", name=f"lnv_{b}{q}")
            act.activation(lnv[:], varv[:], AF.Ln, bias=eps_sb[:, 0:1],
                           scale=1.0)
            act.activation(r_blk[:, s], lnv[:], AF.Exp, bias=0.0, scale=-0.5)
            for c in range(2):
                dve.scalar_tensor_tensor(d_blk[c][:, s], xb[c][:, s], 1.0,
                                         mean[:], OP.mult, OP.subtract,
                                         accum_out=dsum[c][:, col:col + 1])
            # normalized activations for lin1 rhs (ln gamma/beta folded away)
            dve.tensor_mul(zc0[64:128, s], d_blk[0][64:128, s], r_blk[64:128, s])
            dve.tensor_mul(yc1[:, s], d_blk[1][:, s], r_blk[:, s])
        # conv channels 0..63 -> padded fp8 plane (row-aligned chunks)
        for jj in range(NQC):
            sj = slice(jj * QC, (jj + 1) * QC)
            pr = 1 + b * RB + 2 * jj
            dst = zp3[:, pr:pr + 2, 1:1 + W]
            dve.tensor_mul(dst, d_blk[0][0:DC, sj], r_blk[0:DC, sj])

    def stage2(k):
        """pconv + linear1 + gelu for block k (zpad halo rows ready)."""
        zc0, yc1 = zc0_t[k], yc1_t[k]
        wins = [winpool.tile([128, WINR * WP], FP8, tag=f"win{m}",
                             name=f"win{m}_{k}") for m in range(4)]
        win_t[k] = wins
        w3 = [w[:].rearrange("p (r c) -> p r c", c=WP) for w in wins]
        for m in range(4):
            # zero column pads (both edges, all rows), and the junk row that
            # zero-weight DoubleRow taps read past the bottom halo
            pool_e.memset(w3[m][:, :, 0:1], 0.0)
            pool_e.memset(w3[m][:, :, WP - 1:WP], 0.0)
            pool_e.memset(w3[m][:, RB + 2:RB + 3, :], 0.0)
            if k == 0:
                pool_e.memset(w3[m][:, 0:1, 1:1 + W], 0.0)
            if k == NB - 1:
                pool_e.memset(w3[m][:, RB + 1:RB + 2, 1:1 + W], 0.0)

        # partial conv: 9 taps accumulate; evict into zc0 rows 0..63
        for jj in range(NQC):
            pzt = pz.tile([DC, QC], F32, tag="pz", name=f"pz_{k}{jj}")
            r0 = k * RB + 2 * jj
            for t in range(9):
                ky, kx = t // 3 - 1, t % 3 - 1
                rhs = zp3[0:DC, 1 + r0 + ky:1 + r0 + ky + 2, 1 + kx:1 + kx + W]
                te.matmul(pzt[:], pw_sb[t][:], rhs, start=(t == 0), stop=(t == 8))
            act.activation(zc0[0:DC, jj * QC:(jj + 1) * QC], pzt[:], AF.Identity,
                           bias=pc_sb[:, 0:1], scale=1.0 / PSCALE)

        # linear1, h1 half -> fp8 windows (row-aligned), h2 half -> bf16
        for m in range(4):
            for jj in range(NQC):
                ph = pl1.tile([128, QL], tag="ph", name=f"ph_{k}{m}{jj}",
                              dtype=F32)
                sj = slice(jj * QC, (jj + 1) * QC)
                te.matmul(ph[:, :QC], w1_sb[0][:, m * 128:(m + 1) * 128],
                          zc0[:, sj], start=True, stop=False)
                te.matmul(ph[:, :QC], w1_sb[1][:, m * 128:(m + 1) * 128],
                          yc1[:, sj], start=False, stop=True)
                dst = w3[m][:, 1 + 2 * jj:3 + 2 * jj, 1:1 + W]
                act.activation(dst, ph[:, :QC], AF.Gelu, bias=b1_sb[m][:, 0:1],
                               scale=1.0)
        h2s = [h2pool.tile([128, TB], BF16, tag=f"h2_{m}", name=f"h2_{m}_{k}")
               for m in range(4)]
        h2_t[k] = h2s
        for m in range(4):
            for q in range(NQL):
                ph = pl1.tile([128, QL], tag="ph", name=f"ph2_{k}{m}{q}",
                              dtype=F32)
                s = slice(q * QL, (q + 1) * QL)
                te.matmul(ph[:], w1_sb[0][:, (m + 4) * 128:(m + 5) * 128],
                          zc0[:, s], start=True, stop=False)
                te.matmul(ph[:], w1_sb[1][:, (m + 4) * 128:(m + 5) * 128],
                          yc1[:, s], start=False, stop=True)
                act.activation(h2s[m][:, s], ph[:], AF.Gelu,
                               bias=b1_sb[m + 4][:, 0:1], scale=1.0)
        # halo A: first padded row of win_k <- last interior row of win_{k-1}
        if k > 0:
            for m in range(4):
                prev = win_t[k - 1][m][:].rearrange("p (r c) -> p r c", c=WP)
                dve.tensor_copy(w3[m][:, 0:1, :], prev[:, RB:RB + 1, :])

    def stage3(k):
        """dwconv + gelu + product + linear2 + mlp eviction for block k."""
        wins = win_t[k]
        w3 = [w[:].rearrange("p (r c) -> p r c", c=WP) for w in wins]
        if k < NB - 1:
            for m in range(4):
                nxt = win_t[k + 1][m][:].rearrange("p (r c) -> p r c", c=WP)
                dve.tensor_copy(w3[m][:, RB + 1:RB + 2, :], nxt[:, 1:2, :])
        h2s = h2_t[k]
        for jj in range(NQC):
            sj = slice(jj * QC, (jj + 1) * QC)
            prods = []
            for m in range(4):
                pd = pdw.tile([128, QC], F32, tag="pd", name=f"pd_{k}{m}{jj}")
                for p, (taps, delta) in enumerate(DW_PAIRS):
                    (dy, dx) = taps[0]
                    r0 = 2 * jj + 1 + dy          # padded row of first in-row
                    c0 = 1 + dx
                    base = w3[m][:, r0:r0 + 2, c0:c0 + W]
                    part = list(base.ap)[0]
                    rhs = _ap(base, 0, [list(part), [delta, 2], [WP, 2], [1, W]])
                    lhsT = dw_sb[p][m][:].rearrange("k (j m) -> k j m", m=128)
                    te.matmul(pd[:], lhsT, rhs, start=(p == 0), stop=(p == 5),
                              perf_mode=mybir.MatmulPerfMode.DoubleRow)
                h1g = gpool.tile([128, QC], BF16, tag=f"h1g{m}",
                                 name=f"h1g_{k}{m}{jj}")
                act.activation(h1g[:], pd[:], AF.Gelu, bias=db_sb[m][:, 0:1],
                               scale=1.0 / DSCALE)
                prod = gpool.tile([128, QC], BF16, tag=f"prod{m}",
                                  name=f"prod_{k}{m}{jj}")
                if S3L >= 2:
                    dve.tensor_mul(prod[:], h1g[:], h2s[m][:, sj])
                else:
                    dve.tensor_copy(prod[:], h1g[:])
                prods.append(prod)
            if S3L < 3:
                continue
            for mc in range(2):
                pm = pml.tile([128, QC], F32, tag="pm", name=f"pm_{k}{mc}{jj}")
                for kf in range(4):
                    te.matmul(pm[:], w2_sb[kf][:, mc * 128:(mc + 1) * 128],
                              prods[kf][:], start=(kf == 0), stop=(kf == 3))
                mt = mpool.tile([128, QC], BF16, tag=f"mlp{mc}",
                                name=f"mlp_{k}{mc}{jj}")
                col = k * NQC + jj
                dve.tensor_scalar(mt[:], pm[:], b2_sb[mc][:, 0:1], 0.0,
                                  OP.add, OP.add,
                                  accum_out=msum[mc][:, col:col + 1])
                if S3L >= 4:
                    g0 = k * TB + jj * QC
                    sdma.dma_start(mlp_d[mc * 128:(mc + 1) * 128, g0:g0 + QC],
                                   mt[:])

    # ---------------- phase 1: pipelined blocks ----------------
    LVL = int(os.environ.get("K_LVL", "4"))
    for i in range(NB + 2):
        if i < NB and LVL >= 1:
            stage1(i)
        if 0 <= i - 1 < NB and LVL >= 2:
            stage2(i - 1)
        if 0 <= i - 2 < NB and LVL >= 3:
            stage3(i - 2)

    # ---------------- phase 2: SplitAttn tail ----------------
    red = _tile(tc, [128, 8], F32, name="red")
    dve.tensor_reduce(red[:, 0:1], dsum[0][:], mybir.AxisListType.X, OP.add)
    dve.tensor_reduce(red[:, 1:2], dsum[1][:], mybir.AxisListType.X, OP.add)
    dve.tensor_reduce(red[:, 2:3], musum[:], mybir.AxisListType.X, OP.add)
    dve.tensor_reduce(red[:, 3:4], msum[0][:], mybir.AxisListType.X, OP.add)
    dve.tensor_reduce(red[:, 4:5], msum[1][:], mybir.AxisListType.X, OP.add)
    gvec = _tile(tc, [128, 2], F32, name="gvec")
    for c in range(2):
        dve.tensor_add(gvec[:, c:c + 1], red[:, c:c + 1], red[:, 2:3])
        dve.tensor_add(gvec[:, c:c + 1], gvec[:, c:c + 1], red[:, 3 + c:4 + c])
        dve.tensor_scalar_mul(gvec[:, c:c + 1], gvec[:, c:c + 1], 1.0 / T)

    pv = pml.tile([1, C], F32, tag="pm", name="pv")
    for c in range(2):
        te.matmul(pv[:], gvec[:, c:c + 1], f1_sb[c][:], start=(c == 0),
                  stop=(c == 1))
    sc1 = _tile(tc, [1, 8], F32, name="sc1")
    dve.tensor_reduce(sc1[:, 0:1], pv[:], mybir.AxisListType.X, OP.add)
    dve.tensor_scalar_mul(sc1[:, 1:2], sc1[:, 0:1], 1.0 / C)   # mean
    vsq = _tile(tc, [1, C], F32, name="vsq")
    act.activation(vsq[:], pv[:], AF.Square, accum_out=sc1[:, 2:3])
    dve.tensor_mul(sc1[:, 3:4], sc1[:, 1:2], sc1[:, 1:2])      # mean^2
    dve.scalar_tensor_tensor(sc1[:, 4:5], sc1[:, 2:3], 1.0 / C, sc1[:, 3:4],
                             OP.mult, OP.subtract)             # var
    dve.tensor_scalar_add(sc1[:, 5:6], sc1[:, 4:5], LN_EPS)
    dve.reciprocal(sc1[:, 6:7], sc1[:, 5:6])
    act.activation(sc1[:, 7:8], sc1[:, 6:7], AF.Sqrt)          # rstd
    vn = _tile(tc, [1, C], F32, name="vn")
    dve.tensor_scalar(vn[:], pv[:], sc1[:, 1:2], sc1[:, 7:8], OP.subtract,
                      OP.mult)
    dve.tensor_mul(vn[:], vn[:], bg_sb[:])
    dve.tensor_add(vn[:], vn[:], bb_sb[:])
    dve.tensor_scalar_max(vn[:], vn[:], 0.0)
    ggc = _tile(tc, [128, 2], F32, name="ggc")
    for c in range(2):
        sdma.dma_start(ggc[:, c:c + 1], vn[0:1, c * 128:(c + 1) * 128])
    pu = pml.tile([1, C], F32, tag="pm", name="pu")
    for c in range(2):
        te.matmul(pu[:], ggc[:, c:c + 1], f2_sb[c][:], start=(c == 0),
                  stop=(c == 1))
    arow = _tile(tc, [1, C], F32, name="arow")
    act.activation(arow[:], pu[:], AF.Sigmoid)
    acol = _tile(tc, [128, 2], F32, name="acol")
    for c in range(2):
        sdma.dma_start(acol[:, c:c + 1], arow[0:1, c * 128:(c + 1) * 128])

    # ---------------- phase 3: out = (x + mlp) * a ----------------
    TB3 = 768
    for i3 in range(T // TB3):
        g0 = i3 * TB3
        for c in range(2):
            x3 = xpool.tile([128, TB3], F32, tag=f"x{c}", name=f"x3_{c}_{i3}")
            sdma.dma_start(x3[:], xf[c * 128:(c + 1) * 128, g0:g0 + TB3])
            ml = mpool.tile([128, TB3], BF16, tag=f"ml3{c}", name=f"ml_{c}_{i3}")
            sdma.dma_start(ml[:], mlp_d[c * 128:(c + 1) * 128, g0:g0 + TB3])
            mla = mpool.tile([128, TB3], BF16, tag=f"mla3{c}",
                             name=f"mla_{c}_{i3}")
            dve.tensor_scalar_mul(mla[:], ml[:], acol[:, c:c + 1])
            o3 = opool.tile([128, TB3], F32, tag=f"o{c}", name=f"o_{c}_{i3}")
            dve.scalar_tensor_tensor(o3[:], x3[:], acol[:, c:c + 1], mla[:],
                                     OP.mult, OP.add)
            sdma.dma_start(outf[c * 128:(c + 1) * 128, g0:g0 + TB3], o3[:])

    ctx.close()
    perm = _PERM_POOL.pop(id(tc), None)
    if perm is not None:
        perm.release()


# ---------------------------------------------------------------------------
# host-side weight prep + execution
# ---------------------------------------------------------------------------

def _prep_weights(ln2_g, ln2_b, pconv_w, lin1_w, lin1_b, dw_w, dw_b,
                  lin2_w, lin2_b, fc1_w, bn1_g, bn1_b, fc2_w):
    ln2_g = np.asarray(ln2_g, np.float32)
    ln2_b = np.asarray(ln2_b, np.float32)
    lin1_w = np.asarray(lin1_w, np.float32)
    gscale = np.ones(C, np.float32)
    gscale[DC:] = ln2_g[DC:]
    w1p = (lin1_w * gscale[None, :]).T.astype(NPBF16).copy()      # [C, F1]
    b1p = (np.asarray(lin1_b, np.float32)
           + lin1_w[:, DC:] @ ln2_b[DC:]).reshape(F1, 1).astype(np.float32)
    w2p = np.asarray(lin2_w, np.float32).T.astype(NPBF16).copy()  # [HID, C]
    b2p = np.asarray(lin2_b, np.float32).reshape(C, 1).copy()
    pw = np.asarray(pconv_w, np.float32)                          # [3,3,DC,DC]
    pwp = (pw * ln2_g[:DC][None, None, :, None] * PSCALE)
    pwp = pwp.reshape(9, DC, DC).astype(NPFP8).copy()
    pconst = np.einsum('tio,i->o', pw.reshape(9, DC, DC),
                       ln2_b[:DC]).reshape(DC, 1).astype(np.float32)
    dwf = np.asarray(dw_w, np.float32)[:, :, 0, :]                # [3,3,HID]
    dwdr = np.zeros((6, 4, 128, 2, 128), np.float32)
    for p, (taps, _delta) in enumerate(DW_PAIRS):
        for j, tap in enumerate(taps):
            if tap is None:
                continue
            dy, dx = tap
            for m in range(4):
                ch = np.arange(128)
                dwdr[p, m, ch, j, ch] = dwf[dy + 1, dx + 1, m * 128 + ch] * DSCALE
    dwdr = dwdr.reshape(6, 4, 128, 256).astype(NPFP8).copy()
    dbp = np.asarray(dw_b, np.float32).reshape(HID, 1).copy()
    f1t = np.asarray(fc1_w, np.float32).T.copy()                  # [c, f]
    f2t = np.asarray(fc2_w, np.float32).T.copy()                  # [f, c]
    bgp = np.asarray(bn1_g, np.float32).reshape(1, C).copy()
    bbp = np.asarray(bn1_b, np.float32).reshape(1, C).copy()
    return dict(w1t=w1p, b1=b1p, w2t=w2p, b2=b2p, pw=pwp, pconst=pconst,
                dwdr=dwdr, dwb=dbp, fc1t=f1t, fc2t=f2t, bn1g=bgp, bn1b=bbp)


_CACHE = {}


def _get_runner():
    if "runner" in _CACHE:
        return _CACHE["runner"]

    import jax
    from jax.sharding import Mesh, PartitionSpec
    from jax.experimental.shard_map import shard_map
    from concourse import bass2jax
    from concourse.bass2jax import _bass_exec_p, partition_id_tensor

    nc = build_bass()
    bass2jax.install_neuronx_cc_hook()

    partition_name = (nc.partition_id_tensor.name
                      if nc.partition_id_tensor else None)
    in_names, out_names, out_avals, zero_outs = [], [], [], []
    for alloc in nc.m.functions[0].allocations:
        if not isinstance(alloc, mybir.MemoryLocationSet):
            continue
        name = alloc.memorylocations[0].name
        if alloc.kind == "ExternalInput":
            if name != partition_name:
                in_names.append(name)
        elif alloc.kind == "ExternalOutput":
            shape = tuple(alloc.tensor_shape)
            dtype = mybir.dt.np(alloc.dtype)
            out_names.append(name)
            out_avals.append(jax.core.ShapedArray(shape, dtype))
            zero_outs.append(np.zeros(shape, dtype))
    n_params = len(in_names)
    n_outs = len(out_avals)
    all_names = list(in_names) + list(out_names)
    if partition_name is not None:
        all_names.append(partition_name)
    donate = tuple(range(n_params, n_params + n_outs))

    def _body(*args):
        operands = list(args)
        if partition_name is not None:
            operands.append(partition_id_tensor())
        outs = _bass_exec_p.bind(
            *operands, out_avals=tuple(out_avals), in_names=tuple(all_names),
            out_names=tuple(out_names), lowering_input_output_aliases=(),
            sim_require_finite=False, sim_require_nnan=False, nc=nc)
        return tuple(outs)

    devices = jax.devices()[:N_CORES]
    mesh = Mesh(np.asarray(devices), ("core",))
    in_specs = (PartitionSpec("core"),) * (n_params + n_outs)
    out_specs = (PartitionSpec("core"),) * n_outs
    sharded = jax.jit(
        shard_map(_body, mesh=mesh, in_specs=in_specs, out_specs=out_specs,
                  check_rep=False),
        donate_argnums=donate, keep_unused=True)

    runner = dict(fn=sharded, in_names=in_names, out_names=out_names,
                  zero_outs=zero_outs, n_params=n_params)
    _CACHE["runner"] = runner
    return runner


def _run_cores(in_maps):
    import jax
    r = _get_runner()
    per_core = [[np.asarray(m[name]) for name in r["in_names"]]
                for m in in_maps]
    concat_in = [np.concatenate([per_core[c][i] for c in range(N_CORES)], axis=0)
                 for i in range(r["n_params"])]
    concat_zero = [np.concatenate([z] * N_CORES, axis=0)
                   for z in r["zero_outs"]]
    outs = r["fn"](*concat_in, *concat_zero)
    outs = [np.asarray(o) for o in outs]
    results = []
    for c in range(N_CORES):
        d = {}
        for i, name in enumerate(r["out_names"]):
            n0 = r["zero_outs"][i].shape[0]
            d[name] = outs[i][c * n0:(c + 1) * n0]
        results.append(d)
    return results


def _make_in_maps(inputs):
    x = np.asarray(inputs["x"], np.float32)
    wk = {k: v for k, v in inputs.items() if k not in ("x", "record_len")}
    prepped = _prep_weights(**wk)
    in_maps = []
    for b in range(N_CORES):
        m = dict(prepped)
        m["x"] = np.ascontiguousarray(x[b])
        in_maps.append(m)
    return in_maps


def kernel(**inputs):
    in_maps = _make_in_maps(inputs)
    results = _run_cores(in_maps)
    out = np.stack([results[b]["out"] for b in range(N_CORES)], axis=0)
    return out.astype(np.float32)


if __name__ == "__main__":
    xs = {"x": np.random.randn(8, C, H, W).astype(np.float32)}
    print("building only (smoke)...")
    nc = build_bass()
    print("built OK")


# revision 37
# speedup vs baseline: 9059.9576x; 9059.9576x over previous
"""Trainium2 Bass kernel for nn_Enhancer_63350767616202.

Data-parallel over batch (8 samples -> 8 cores). Per core, everything stays in
channel-major [C, T] layout (matches the input's [C, H, W] memory layout), so
the two big linear layers, the partial 3x3 conv and the depthwise 3x3 conv all
run on the tensor engine with zero transposes:

  phase 1 (streamed over 12 row-blocks of 8 rows):
    LN stats via bf16 ones-matmul reductions (M=128 -> broadcast for free)
    y = (x - mu) * rsqrt(var+eps)        [ln gamma/beta folded into weights]
    pconv   : 9 accumulating matmuls over a zero-padded fp8 plane
    linear1 : bf16 matmuls, Gelu eviction on ACT (h1 -> padded fp8 windows)
    dwconv  : fp8 DoubleRow diagonal matmuls (2 taps per matmul)
    linear2 : bf16 matmuls over gelu(dwconv)*h2, evicted to a DRAM scratch
    channel sums of x and mlp accumulated on the fly (accum_out)
  phase 2: SplitAttn tail on [256]-vectors (fc1 -> LN -> relu -> fc2 -> sigmoid)
  phase 3: out = (x + mlp) * a   (restreamed)
"""

import os
import sys

for _p in ("/opt/trn_rl_repo", "/root/.axon_site/_ro/trn_rl_repo"):
    if os.path.isdir(_p) and _p not in sys.path:
        sys.path.append(_p)

import numpy as np
import ml_dtypes

import concourse.bass as bass
import concourse.mybir as mybir
import concourse.tile as tile
from concourse import bacc
from concourse.tile import TileContext

F32 = mybir.dt.float32
F32R = mybir.dt.float32r
BF16 = mybir.dt.bfloat16
FP8 = mybir.dt.float8e4
AF = mybir.ActivationFunctionType
OP = mybir.AluOpType

NPBF16 = ml_dtypes.bfloat16
NPFP8 = ml_dtypes.float8_e4m3

C = 256
H, W = 96, 192
T = H * W
HID = 512
F1 = 1024
DC = 64          # partial conv channels
LN_EPS = 1e-5

RB = 8           # rows per block
TB = RB * W      # tokens per block (1536)
NB = H // RB     # 12 blocks
QL = 512         # linear-chunk tokens
NQL = TB // QL   # 3
QC = 384         # conv-chunk tokens (2 rows)
NQC = TB // QC   # 4
WP = W + 2       # padded width 194
ZR = H + 3       # padded rows for pconv plane (+1 junk row)
WINR = RB + 3    # padded rows per window (8 + halo x2 + junk row)

PSCALE = 64.0    # fp8 weight scaling for pconv
DSCALE = 64.0    # fp8 weight scaling for dwconv

# dwconv DoubleRow tap pairs: ((dy0,dx0),(dy1,dx1)|None). The pair delta is
# always +WP (one padded row down): the device crashes on negative or
# overlapping pair strides. Singles carry a zero-weight second tap that reads
# the zeroed junk row below the window.
DW_PAIRS = [
    (((-1, -1), (0, -1)), WP),
    (((-1, 0), (0, 0)), WP),
    (((-1, 1), (0, 1)), WP),
    (((1, -1), (1, 1)), 2),
    (((1, 0), None), WP),
]

N_CORES = 8


def _ap(base, offset_delta, ap_dims):
    """Build a raw AP on base's tensor with extra offset and explicit dims."""
    return bass.AP(tensor=base.tensor, offset=base.offset + offset_delta,
                   ap=ap_dims)


def build_bass():
    nc = bacc.Bacc("TRN2", target_bir_lowering=False, debug=False,
                   num_devices=N_CORES)

    # ---- per-core I/O ----
    x_d = nc.dram_tensor("x", [C, H, W], F32, kind="ExternalInput")
    w1_d = nc.dram_tensor("w1t", [C, F1], BF16, kind="ExternalInput")
    b1_d = nc.dram_tensor("b1", [F1, 1], F32, kind="ExternalInput")
    w2_d = nc.dram_tensor("w2t", [HID, C], BF16, kind="ExternalInput")
    b2_d = nc.dram_tensor("b2", [C, 1], F32, kind="ExternalInput")
    pw_d = nc.dram_tensor("pw", [5, DC, 2 * DC], FP8, kind="ExternalInput")
    pc_d = nc.dram_tensor("pconst", [DC, 1], F32, kind="ExternalInput")
    dw_d = nc.dram_tensor("dwdr", [5, 4, 128, 2 * 128], FP8, kind="ExternalInput")
    db_d = nc.dram_tensor("dwb", [HID, 1], F32, kind="ExternalInput")
    f1_d = nc.dram_tensor("fc1t", [C, C], F32, kind="ExternalInput")
    f2_d = nc.dram_tensor("fc2t", [C, C], F32, kind="ExternalInput")
    bg_d = nc.dram_tensor("bn1g", [1, C], F32, kind="ExternalInput")
    bb_d = nc.dram_tensor("bn1b", [1, C], F32, kind="ExternalInput")
    out_d = nc.dram_tensor("out", [C, H, W], F32, kind="ExternalOutput")

    xf = x_d[:].rearrange("c h w -> c (h w)")
    outf = out_d[:].rearrange("c h w -> c (h w)")

    with TileContext(nc) as tc:
        _build_body(nc, tc, xf, outf, w1_d, b1_d, w2_d, b2_d, pw_d, pc_d,
                    dw_d, db_d, f1_d, f2_d, bg_d, bb_d)

    nc.compile()
    return nc


_PERM_POOL = {}


def _tile(tc, shape, dtype, name):
    pool = _PERM_POOL.get(id(tc))
    if pool is None:
        pool = tc.alloc_tile_pool(name="perm", bufs=1)
        _PERM_POOL[id(tc)] = pool
    return pool.tile(shape, dtype, name=name, tag=name)


def _build_body(nc, tc, xf, outf, w1_d, b1_d, w2_d, b2_d, pw_d, pc_d,
                dw_d, db_d, f1_d, f2_d, bg_d, bb_d):
    act, dve, pool_e, te, sdma = nc.scalar, nc.vector, nc.gpsimd, nc.tensor, nc.sync

    # ---------------- persistent tiles ----------------
    w1_sb = [_tile(tc, [128, F1], BF16, name=f"w1_{i}") for i in range(2)]
    w2_sb = [_tile(tc, [128, C], BF16, name=f"w2_{i}") for i in range(4)]
    pw_sb = [_tile(tc, [DC, 2 * DC], FP8, name=f"pw_{t}") for t in range(5)]
    dw_sb = [[_tile(tc, [128, 256], FP8, name=f"dw_{p}_{m}") for m in range(4)]
             for p in range(5)]
    b1_sb = [_tile(tc, [128, 1], F32, name=f"b1_{m}") for m in range(8)]
    b2_sb = [_tile(tc, [128, 1], F32, name=f"b2_{m}") for m in range(2)]
    db_sb = [_tile(tc, [128, 1], F32, name=f"db_{m}") for m in range(4)]
    pc_sb = _tile(tc, [DC, 1], F32, name="pc_sb")
    f1_sb = [_tile(tc, [128, C], F32, name=f"f1_{i}") for i in range(2)]
    f2_sb = [_tile(tc, [128, C], F32, name=f"f2_{i}") for i in range(2)]
    bg_sb = _tile(tc, [1, C], F32, name="bg_sb")
    bb_sb = _tile(tc, [1, C], F32, name="bb_sb")
    ones_b = _tile(tc, [128, 128], BF16, name="ones_b")
    eps_sb = _tile(tc, [128, 1], F32, name="eps_sb")
    zpad = _tile(tc, [DC, ZR * WP], FP8, name="zpad")

    dsum = [_tile(tc, [128, NB * NQL], F32, name=f"dsum{i}") for i in range(2)]
    msum = [_tile(tc, [128, NB * NQC], F32, name=f"msum{i}") for i in range(2)]

    for i in range(2):
        sdma.dma_start(w1_sb[i][:], w1_d[i * 128:(i + 1) * 128, :])
        sdma.dma_start(f1_sb[i][:], f1_d[i * 128:(i + 1) * 128, :])
        sdma.dma_start(f2_sb[i][:], f2_d[i * 128:(i + 1) * 128, :])
        sdma.dma_start(b2_sb[i][:], b2_d[i * 128:(i + 1) * 128, :])
    for i in range(4):
        sdma.dma_start(w2_sb[i][:], w2_d[i * 128:(i + 1) * 128, :])
        sdma.dma_start(db_sb[i][:], db_d[i * 128:(i + 1) * 128, :])
    for t in range(5):
        sdma.dma_start(pw_sb[t][:], pw_d[t, :, :])
    for p in range(5):
        for m in range(4):
            sdma.dma_start(dw_sb[p][m][:], dw_d[p, m, :, :])
    for m in range(8):
        sdma.dma_start(b1_sb[m][:], b1_d[m * 128:(m + 1) * 128, :])
    sdma.dma_start(pc_sb[:], pc_d[:, :])
    sdma.dma_start(bg_sb[:], bg_d[:, :])
    sdma.dma_start(bb_sb[:], bb_d[:, :])
    pool_e.memset(ones_b[:], 1.0)
    pool_e.memset(eps_sb[:], LN_EPS)
    zp3i = zpad[:].rearrange("p (r c) -> p r c", c=WP)
    pool_e.memset(zp3i[:, 0:1, :], 0.0)            # top pad row
    pool_e.memset(zp3i[:, ZR - 2:ZR, :], 0.0)      # bottom pad + junk rows
    pool_e.memset(zp3i[:, :, 0:1], 0.0)            # left pad col
    pool_e.memset(zp3i[:, :, WP - 1:WP], 0.0)      # right pad col

    zp3 = zpad[:].rearrange("p (r c) -> p r c", c=WP)

    # ---------------- pools ----------------
    import contextlib
    ctx = contextlib.ExitStack()
    xpool = ctx.enter_context(tc.tile_pool(name="xpool", bufs=2))
    spool = ctx.enter_context(tc.tile_pool(name="spool", bufs=2))
    ypool = ctx.enter_context(tc.tile_pool(name="ypool", bufs=2))
    winpool = ctx.enter_context(tc.tile_pool(name="winpool", bufs=2))
    h2pool = ctx.enter_context(tc.tile_pool(name="h2pool", bufs=2))
    gpool = ctx.enter_context(tc.tile_pool(name="gpool", bufs=2))
    mpool = ctx.enter_context(tc.tile_pool(name="mpool", bufs=2))
    opool = ctx.enter_context(tc.tile_pool(name="opool", bufs=2))
    dpool = ctx.enter_context(tc.tile_pool(name="drampool", bufs=1, space="DRAM"))

    pstat = ctx.enter_context(tc.tile_pool(name="pstat", bufs=1, space="PSUM"))
    pl1 = ctx.enter_context(tc.tile_pool(name="pl1", bufs=2, space="PSUM"))
    pz = ctx.enter_context(tc.tile_pool(name="pz", bufs=1, space="PSUM"))
    pdw = ctx.enter_context(tc.tile_pool(name="pdw", bufs=2, space="PSUM"))
    pml = ctx.enter_context(tc.tile_pool(name="pml", bufs=1, space="PSUM"))

    mlp_d = dpool.tile([C, T], BF16, name="mlp_scratch")

    # state carried across pipeline iterations
    zc0_t, yc1_t = {}, {}            # lin1 rhs tiles per block
    win_t = {}                       # win_t[k] = [4 tiles]
    h2_t = {}                        # h2_t[k] = [4 tiles]

    def stage1(b):
        """LN stats + normalized activations for block b."""
        g0 = b * TB
        xb = [xpool.tile([128, TB], F32, tag=f"x{c}", name=f"xb{c}_{b}")
              for c in range(2)]
        for c in range(2):
            sdma.dma_start(xb[c][:], xf[c * 128:(c + 1) * 128, g0:g0 + TB])

        r_blk = spool.tile([128, TB], BF16, tag="r", name=f"r_{b}")
        varb = spool.tile([128, TB], BF16, tag="var", name=f"var_{b}")
        d_blk = [spool.tile([128, TB], BF16, tag=f"d{c}", name=f"d{c}_{b}")
                 for c in range(2)]
        zc0 = ypool.tile([128, TB], BF16, tag="zc0", name=f"zc0_{b}")
        yc1 = ypool.tile([128, TB], BF16, tag="yc1", name=f"yc1_{b}")
        zc0_t[b], yc1_t[b] = zc0, yc1

        for q in range(NQL):
            s = slice(q * QL, (q + 1) * QL)
            col = b * NQL + q
            # x^2 (ACT), then ones-matmul stat reductions in f32r
            xcv = [spool.tile([128, QL], BF16, tag=f"cv{c}", name=f"cv{c}_{b}{q}")
                   for c in range(2)]
            xsq = [spool.tile([128, QL], BF16, tag=f"sq{c}", name=f"sq{c}_{b}{q}")
                   for c in range(2)]
            for c in range(2):
                dve.tensor_scalar(xcv[c][:], xb[c][:, s], 1.0, 0.0, OP.mult,
                                  OP.add, accum_out=dsum[c][:, col:col + 1])
                pool_e.tensor_mul(xsq[c][:], xb[c][:, s], xb[c][:, s])
            pmu = pstat.tile([128, QL], F32, tag="pmu", name=f"pmu_{b}{q}")
            psq = pstat.tile([128, QL], F32, tag="psq", name=f"psq_{b}{q}")
            for c in range(2):
                te.matmul(pmu[:], ones_b[:], xcv[c][:],
                          start=(c == 0), stop=(c == 1))
                te.matmul(psq[:], ones_b[:], xsq[c][:],
                          start=(c == 0), stop=(c == 1))
            mean = spool.tile([128, QL], BF16, tag="mean", name=f"mean_{b}{q}")
            dve.tensor_scalar_mul(mean[:], pmu[:], 1.0 / C)
            msq = spool.tile([128, QL], BF16, tag="msq", name=f"msq_{b}{q}")
            dve.tensor_mul(msq[:], mean[:], mean[:])
            dve.scalar_tensor_tensor(varb[:, s], psq[:], 1.0 / C, msq[:],
                                     OP.mult, OP.subtract)
            for c in range(2):
                pool_e.tensor_sub(d_blk[c][:, s], xb[c][:, s], mean[:])
        act.activation(r_blk[:], varb[:], AF.Abs_reciprocal_sqrt,
                       bias=eps_sb[:, 0:1], scale=1.0)
        for q in range(NQL):
            s = slice(q * QL, (q + 1) * QL)
            # normalized activations for lin1 rhs (ln gamma/beta folded away)
            dve.tensor_mul(zc0[64:128, s], d_blk[0][64:128, s], r_blk[64:128, s])
            dve.tensor_mul(yc1[:, s], d_blk[1][:, s], r_blk[:, s])
        # conv channels 0..63 -> padded fp8 plane (row-aligned chunks)
        for jj in range(NQC):
            sj = slice(jj * QC, (jj + 1) * QC)
            pr = 1 + b * RB + 2 * jj
            dst = zp3[:, pr:pr + 2, 1:1 + W]
            dve.tensor_mul(dst, d_blk[0][0:DC, sj], r_blk[0:DC, sj])

    def stage2(k):
        """pconv + linear1 + gelu for block k (zpad halo rows ready)."""
        zc0, yc1 = zc0_t[k], yc1_t[k]
        wins = [winpool.tile([128, WINR * WP], FP8, tag=f"win{m}",
                             name=f"win{m}_{k}") for m in range(4)]
        win_t[k] = wins
        w3 = [w[:].rearrange("p (r c) -> p r c", c=WP) for w in wins]
        for m in range(4):
            # zero column pads (both edges, all rows), and the junk row that
            # zero-weight DoubleRow taps read past the bottom halo
            pool_e.memset(w3[m][:, :, 0:1], 0.0)
            pool_e.memset(w3[m][:, :, WP - 1:WP], 0.0)
            pool_e.memset(w3[m][:, RB + 2:RB + 3, :], 0.0)
            if k == 0:
                pool_e.memset(w3[m][:, 0:1, 1:1 + W], 0.0)
            if k == NB - 1:
                pool_e.memset(w3[m][:, RB + 1:RB + 2, 1:1 + W], 0.0)

        # partial conv: 9 taps accumulate; evict into zc0 rows 0..63
        for jj in range(NQC):
            pzt = pz.tile([DC, QC], F32, tag="pz", name=f"pz_{k}{jj}")
            r0 = k * RB + 2 * jj
            for t, (taps, delta) in enumerate(DW_PAIRS):
                dy, dx = taps[0]
                base = zp3[0:DC, 1 + r0 + dy:1 + r0 + dy + 2, 1 + dx:1 + dx + W]
                part = list(base.ap)[0]
                rhs = _ap(base, 0, [list(part), [delta, 2], [WP, 2], [1, W]])
                lhsT = pw_sb[t][:].rearrange("k (j m) -> k j m", m=DC)
                te.matmul(pzt[:], lhsT, rhs, start=(t == 0), stop=(t == 4),
                          perf_mode=mybir.MatmulPerfMode.DoubleRow)
            dve.tensor_scalar(zc0[0:DC, jj * QC:(jj + 1) * QC], pzt[:],
                              1.0 / PSCALE, pc_sb[:, 0:1], OP.mult, OP.add)

        # linear1, h1 half -> fp8 windows (row-aligned), h2 half -> bf16
        for m in range(4):
            for jj in range(NQC):
                ph = pl1.tile([128, QL], tag="ph", name=f"ph_{k}{m}{jj}",
                              dtype=F32)
                sj = slice(jj * QC, (jj + 1) * QC)
                te.matmul(ph[:, :QC], w1_sb[0][:, m * 128:(m + 1) * 128],
                          zc0[:, sj], start=True, stop=False)
                te.matmul(ph[:, :QC], w1_sb[1][:, m * 128:(m + 1) * 128],
                          yc1[:, sj], start=False, stop=True)
                dst = w3[m][:, 1 + 2 * jj:3 + 2 * jj, 1:1 + W]
                act.activation(dst, ph[:, :QC], AF.Gelu, bias=b1_sb[m][:, 0:1],
                               scale=1.0)
        h2s = [h2pool.tile([128, TB], BF16, tag=f"h2_{m}", name=f"h2_{m}_{k}")
               for m in range(4)]
        h2_t[k] = h2s
        for m in range(4):
            for q in range(NQL):
                ph = pl1.tile([128, QL], tag="ph", name=f"ph2_{k}{m}{q}",
                              dtype=F32)
                s = slice(q * QL, (q + 1) * QL)
                te.matmul(ph[:], w1_sb[0][:, (m + 4) * 128:(m + 5) * 128],
                          zc0[:, s], start=True, stop=False)
                te.matmul(ph[:], w1_sb[1][:, (m + 4) * 128:(m + 5) * 128],
                          yc1[:, s], start=False, stop=True)
                act.activation(h2s[m][:, s], ph[:], AF.Gelu,
                               bias=b1_sb[m + 4][:, 0:1], scale=1.0)
        # halo A: first padded row of win_k <- last interior row of win_{k-1}
        if k > 0:
            for m in range(4):
                prev = win_t[k - 1][m][:].rearrange("p (r c) -> p r c", c=WP)
                pool_e.tensor_copy(w3[m][:, 0:1, :], prev[:, RB:RB + 1, :])

    def stage3(k):
        """dwconv + gelu + product + linear2 + mlp eviction for block k."""
        wins = win_t[k]
        mlpt = [mpool.tile([128, TB], BF16, tag=f"mlp{mc}", name=f"mlp_{k}{mc}")
                for mc in range(2)]
        w3 = [w[:].rearrange("p (r c) -> p r c", c=WP) for w in wins]
        if k < NB - 1:
            for m in range(4):
                nxt = win_t[k + 1][m][:].rearrange("p (r c) -> p r c", c=WP)
                pool_e.tensor_copy(w3[m][:, RB + 1:RB + 2, :], nxt[:, 1:2, :])
        h2s = h2_t[k]
        for jj in range(NQC):
            sj = slice(jj * QC, (jj + 1) * QC)
            prods = []
            for m in range(4):
                pd = pdw.tile([128, QC], F32, tag="pd", name=f"pd_{k}{m}{jj}")
                for p, (taps, delta) in enumerate(DW_PAIRS):
                    (dy, dx) = taps[0]
                    r0 = 2 * jj + 1 + dy          # padded row of first in-row
                    c0 = 1 + dx
                    base = w3[m][:, r0:r0 + 2, c0:c0 + W]
                    part = list(base.ap)[0]
                    rhs = _ap(base, 0, [list(part), [delta, 2], [WP, 2], [1, W]])
                    lhsT = dw_sb[p][m][:].rearrange("k (j m) -> k j m", m=128)
                    te.matmul(pd[:], lhsT, rhs, start=(p == 0), stop=(p == 4),
                              perf_mode=mybir.MatmulPerfMode.DoubleRow)
                h1g = gpool.tile([128, QC], BF16, tag=f"h1g{m}",
                                 name=f"h1g_{k}{m}{jj}")
                act.activation(h1g[:], pd[:], AF.Gelu, bias=db_sb[m][:, 0:1],
                               scale=1.0 / DSCALE)
                prod = gpool.tile([128, QC], BF16, tag=f"prod{m}",
                                  name=f"prod_{k}{m}{jj}")
                dve.tensor_mul(prod[:], h1g[:], h2s[m][:, sj])
                prods.append(prod)
            for mc in range(2):
                pm = pml.tile([128, QC], F32, tag="pm", name=f"pm_{k}{mc}{jj}")
                for kf in range(4):
                    te.matmul(pm[:], w2_sb[kf][:, mc * 128:(mc + 1) * 128],
                              prods[kf][:], start=(kf == 0), stop=(kf == 3))
                col = k * NQC + jj
                dve.tensor_scalar(mlpt[mc][:, jj * QC:(jj + 1) * QC], pm[:],
                                  b2_sb[mc][:, 0:1], 0.0, OP.add, OP.add,
                                  accum_out=msum[mc][:, col:col + 1])
        for mc in range(2):
            sdma.dma_start(mlp_d[mc * 128:(mc + 1) * 128, k * TB:(k + 1) * TB],
                           mlpt[mc][:])

    # ---------------- phase 1: pipelined blocks ----------------
    for i in range(NB + 2):
        if i < NB:
            stage1(i)
        if 0 <= i - 1 < NB:
            stage2(i - 1)
        if 0 <= i - 2 < NB:
            stage3(i - 2)

    # ---------------- phase 2: SplitAttn tail ----------------
    red = _tile(tc, [128, 8], F32, name="red")
    dve.tensor_reduce(red[:, 0:1], dsum[0][:], mybir.AxisListType.X, OP.add)
    dve.tensor_reduce(red[:, 1:2], dsum[1][:], mybir.AxisListType.X, OP.add)
    dve.tensor_reduce(red[:, 3:4], msum[0][:], mybir.AxisListType.X, OP.add)
    dve.tensor_reduce(red[:, 4:5], msum[1][:], mybir.AxisListType.X, OP.add)
    gvec = _tile(tc, [128, 2], F32, name="gvec")
    for c in range(2):
        dve.tensor_add(gvec[:, c:c + 1], red[:, c:c + 1], red[:, 3 + c:4 + c])
        dve.tensor_scalar_mul(gvec[:, c:c + 1], gvec[:, c:c + 1], 1.0 / T)

    pv = pml.tile([1, C], F32, tag="pm", name="pv")
    for c in range(2):
        te.matmul(pv[:], gvec[:, c:c + 1], f1_sb[c][:], start=(c == 0),
                  stop=(c == 1))
    sc1 = _tile(tc, [1, 8], F32, name="sc1")
    dve.tensor_reduce(sc1[:, 0:1], pv[:], mybir.AxisListType.X, OP.add)
    dve.tensor_scalar_mul(sc1[:, 1:2], sc1[:, 0:1], 1.0 / C)   # mean
    vsq = _tile(tc, [1, C], F32, name="vsq")
    act.activation(vsq[:], pv[:], AF.Square, accum_out=sc1[:, 2:3])
    dve.tensor_mul(sc1[:, 3:4], sc1[:, 1:2], sc1[:, 1:2])      # mean^2
    dve.scalar_tensor_tensor(sc1[:, 4:5], sc1[:, 2:3], 1.0 / C, sc1[:, 3:4],
                             OP.mult, OP.subtract)             # var
    dve.tensor_scalar_add(sc1[:, 5:6], sc1[:, 4:5], LN_EPS)
    dve.reciprocal(sc1[:, 6:7], sc1[:, 5:6])
    act.activation(sc1[:, 7:8], sc1[:, 6:7], AF.Sqrt)          # rstd
    vn = _tile(tc, [1, C], F32, name="vn")
    dve.tensor_scalar(vn[:], pv[:], sc1[:, 1:2], sc1[:, 7:8], OP.subtract,
                      OP.mult)
    dve.tensor_mul(vn[:], vn[:], bg_sb[:])
    dve.tensor_add(vn[:], vn[:], bb_sb[:])
    dve.tensor_scalar_max(vn[:], vn[:], 0.0)
    ggc = _tile(tc, [128, 2], F32, name="ggc")
    for c in range(2):
        sdma.dma_start(ggc[:, c:c + 1], vn[0:1, c * 128:(c + 1) * 128])
    pu = pml.tile([1, C], F32, tag="pm", name="pu")
    for c in range(2):
        te.matmul(pu[:], ggc[:, c:c + 1], f2_sb[c][:], start=(c == 0),
                  stop=(c == 1))
    arow = _tile(tc, [1, C], F32, name="arow")
    act.activation(arow[:], pu[:], AF.Sigmoid)
    acol = _tile(tc, [128, 2], F32, name="acol")
    for c in range(2):
        sdma.dma_start(acol[:, c:c + 1], arow[0:1, c * 128:(c + 1) * 128])

    # ---------------- phase 3: out = (x + mlp) * a ----------------
    # phase-1-only pools release their SBUF so phase 3 can buffer deeply
    ctx.close()
    ctx3 = contextlib.ExitStack()
    x3pool = ctx3.enter_context(tc.tile_pool(name="x3pool", bufs=3))
    m3pool = ctx3.enter_context(tc.tile_pool(name="m3pool", bufs=3))
    o3pool = ctx3.enter_context(tc.tile_pool(name="o3pool", bufs=3))
    TB3 = 2304
    for i3 in range(T // TB3):
        g0 = i3 * TB3
        for c in range(2):
            x3 = x3pool.tile([128, TB3], F32, tag=f"x{c}", name=f"x3_{c}_{i3}")
            sdma.dma_start(x3[:], xf[c * 128:(c + 1) * 128, g0:g0 + TB3])
            ml = m3pool.tile([128, TB3], BF16, tag=f"ml3{c}",
                             name=f"ml_{c}_{i3}")
            sdma.dma_start(ml[:], mlp_d[c * 128:(c + 1) * 128, g0:g0 + TB3])
            dve.tensor_scalar_mul(ml[:], ml[:], acol[:, c:c + 1])
            o3 = o3pool.tile([128, TB3], F32, tag=f"o{c}", name=f"o_{c}_{i3}")
            dve.scalar_tensor_tensor(o3[:], x3[:], acol[:, c:c + 1], ml[:],
                                     OP.mult, OP.add)
            act.dma_start(outf[c * 128:(c + 1) * 128, g0:g0 + TB3], o3[:])

    ctx3.close()
    perm = _PERM_POOL.pop(id(tc), None)
    if perm is not None:
        perm.release()


# ---------------------------------------------------------------------------
# host-side weight prep + execution
# ---------------------------------------------------------------------------

def _prep_weights(ln2_g, ln2_b, pconv_w, lin1_w, lin1_b, dw_w, dw_b,
                  lin2_w, lin2_b, fc1_w, bn1_g, bn1_b, fc2_w):
    ln2_g = np.asarray(ln2_g, np.float32)
    ln2_b = np.asarray(ln2_b, np.float32)
    lin1_w = np.asarray(lin1_w, np.float32)
    gscale = np.ones(C, np.float32)
    gscale[DC:] = ln2_g[DC:]
    w1p = (lin1_w * gscale[None, :]).T.astype(NPBF16).copy()      # [C, F1]
    b1p = (np.asarray(lin1_b, np.float32)
           + lin1_w[:, DC:] @ ln2_b[DC:]).reshape(F1, 1).astype(np.float32)
    w2p = np.asarray(lin2_w, np.float32).T.astype(NPBF16).copy()  # [HID, C]
    b2p = np.asarray(lin2_b, np.float32).reshape(C, 1).copy()
    pw = np.asarray(pconv_w, np.float32)                          # [3,3,DC,DC]
    pwg = pw * ln2_g[:DC][None, None, :, None] * PSCALE
    pwp = np.zeros((5, DC, 2, DC), np.float32)
    for t, (taps, _delta) in enumerate(DW_PAIRS):
        for j, tap in enumerate(taps):
            if tap is None:
                continue
            dy, dx = tap
            pwp[t, :, j, :] = pwg[dy + 1, dx + 1]
    pwp = pwp.reshape(5, DC, 2 * DC).astype(NPFP8).copy()
    pconst = np.einsum('tio,i->o', pw.reshape(9, DC, DC),
                       ln2_b[:DC]).reshape(DC, 1).astype(np.float32)
    dwf = np.asarray(dw_w, np.float32)[:, :, 0, :]                # [3,3,HID]
    dwdr = np.zeros((5, 4, 128, 2, 128), np.float32)
    for p, (taps, _delta) in enumerate(DW_PAIRS):
        for j, tap in enumerate(taps):
            if tap is None:
                continue
            dy, dx = tap
            for m in range(4):
                ch = np.arange(128)
                dwdr[p, m, ch, j, ch] = dwf[dy + 1, dx + 1, m * 128 + ch] * DSCALE
    dwdr = dwdr.reshape(5, 4, 128, 256).astype(NPFP8).copy()
    dbp = np.asarray(dw_b, np.float32).reshape(HID, 1).copy()
    f1t = np.asarray(fc1_w, np.float32).T.copy()                  # [c, f]
    f2t = np.asarray(fc2_w, np.float32).T.copy()                  # [f, c]
    bgp = np.asarray(bn1_g, np.float32).reshape(1, C).copy()
    bbp = np.asarray(bn1_b, np.float32).reshape(1, C).copy()
    return dict(w1t=w1p, b1=b1p, w2t=w2p, b2=b2p, pw=pwp, pconst=pconst,
                dwdr=dwdr, dwb=dbp, fc1t=f1t, fc2t=f2t, bn1g=bgp, bn1b=bbp)


_CACHE = {}


def _get_runner():
    if "runner" in _CACHE:
        return _CACHE["runner"]

    import jax
    from jax.sharding import Mesh, PartitionSpec
    from jax.experimental.shard_map import shard_map
    from concourse import bass2jax
    from concourse.bass2jax import _bass_exec_p, partition_id_tensor

    nc = build_bass()
    bass2jax.install_neuronx_cc_hook()

    partition_name = (nc.partition_id_tensor.name
                      if nc.partition_id_tensor else None)
    in_names, out_names, out_avals, zero_outs = [], [], [], []
    for alloc in nc.m.functions[0].allocations:
        if not isinstance(alloc, mybir.MemoryLocationSet):
            continue
        name = alloc.memorylocations[0].name
        if alloc.kind == "ExternalInput":
            if name != partition_name:
                in_names.append(name)
        elif alloc.kind == "ExternalOutput":
            shape = tuple(alloc.tensor_shape)
            dtype = mybir.dt.np(alloc.dtype)
            out_names.append(name)
            out_avals.append(jax.core.ShapedArray(shape, dtype))
            zero_outs.append(np.zeros(shape, dtype))
    n_params = len(in_names)
    n_outs = len(out_avals)
    all_names = list(in_names) + list(out_names)
    if partition_name is not None:
        all_names.append(partition_name)
    donate = tuple(range(n_params, n_params + n_outs))

    def _body(*args):
        operands = list(args)
        if partition_name is not None:
            operands.append(partition_id_tensor())
        outs = _bass_exec_p.bind(
            *operands, out_avals=tuple(out_avals), in_names=tuple(all_names),
            out_names=tuple(out_names), lowering_input_output_aliases=(),
            sim_require_finite=False, sim_require_nnan=False, nc=nc)
        return tuple(outs)

    devices = jax.devices()[:N_CORES]
    mesh = Mesh(np.asarray(devices), ("core",))
    in_specs = (PartitionSpec("core"),) * (n_params + n_outs)
    out_specs = (PartitionSpec("core"),) * n_outs
    sharded = jax.jit(
        shard_map(_body, mesh=mesh, in_specs=in_specs, out_specs=out_specs,
                  check_rep=False),
        donate_argnums=donate, keep_unused=True)

    runner = dict(fn=sharded, in_names=in_names, out_names=out_names,
                  zero_outs=zero_outs, n_params=n_params)
    _CACHE["runner"] = runner
    return runner


def _run_cores(in_maps):
    import jax
    r = _get_runner()
    per_core = [[np.asarray(m[name]) for name in r["in_names"]]
                for m in in_maps]
    concat_in = [np.concatenate([per_core[c][i] for c in range(N_CORES)], axis=0)
                 for i in range(r["n_params"])]
    concat_zero = [np.concatenate([z] * N_CORES, axis=0)
                   for z in r["zero_outs"]]
    outs = r["fn"](*concat_in, *concat_zero)
    outs = [np.asarray(o) for o in outs]
    results = []
    for c in range(N_CORES):
        d = {}
        for i, name in enumerate(r["out_names"]):
            n0 = r["zero_outs"][i].shape[0]
            d[name] = outs[i][c * n0:(c + 1) * n0]
        results.append(d)
    return results


def _make_in_maps(inputs):
    x = np.asarray(inputs["x"], np.float32)
    wk = {k: v for k, v in inputs.items() if k not in ("x", "record_len")}
    prepped = _prep_weights(**wk)
    in_maps = []
    for b in range(N_CORES):
        m = dict(prepped)
        m["x"] = np.ascontiguousarray(x[b])
        in_maps.append(m)
    return in_maps


def kernel(**inputs):
    in_maps = _make_in_maps(inputs)
    results = _run_cores(in_maps)
    out = np.stack([results[b]["out"] for b in range(N_CORES)], axis=0)
    return out.astype(np.float32)


if __name__ == "__main__":
    xs = {"x": np.random.randn(8, C, H, W).astype(np.float32)}
    print("building only (smoke)...")
    nc = build_bass()
    print("built OK")
